# revision 5
# baseline (speedup 1.0000x reference)
"""Block-sparse attention Trainium2 kernel (v3, bf16 end-to-end).

Reference: nn.MultiheadAttention-style block-sparse attention, B=1, L=4096,
D=1024, H=16, head_dim=64, block=128, global blocks {0, 24}.

Sharding: head-parallel across 8 cores (2 heads/core); host sums the 8
partial out-projections (upcast from bf16).

v3 over v2:
- bf16 everywhere on the wide paths: x, weights, q/k/v, exp tiles, otr, out.
  PSUM accumulation stays f32. Halves DMA traffic and removes all
  f32->f32r DVE cast copies (34us of DVE in v2).
- x loaded once as 8 resident [128, 4096] bf16 tiles (8 big DMAs).
- Diagonal score blocks batched: 4 matmuls into one [128,512] PSUM tile,
  one exp per chunk-half instead of four.
- Chunk emission interleaved with QKV quads so Act-heavy chunk work
  overlaps PE-heavy projection work.
- Output written bf16, one batched DMA per 512-row chunk via rearranged
  DRAM access pattern.
"""

import sys

sys.path.insert(0, "/opt/trn_rl_repo")
import numpy as np

D = 1024
L = 4096
H = 16
HD = 64
NB = 32
GLOB = (0, 24)
P = 128
SCALE = 1.0 / 8.0

_CACHE = {}


def _build_nc(reps=1):
    import contextlib

    import concourse.mybir as mybir
    import concourse.tile as tile
    from concourse import bacc
    from concourse.masks import make_identity

    f32 = mybir.dt.float32
    f32r = mybir.dt.float32r
    bf16 = mybir.dt.bfloat16
    Act = mybir.ActivationFunctionType
    AluMult = mybir.AluOpType.mult

    nc = bacc.Bacc("TRN2", target_bir_lowering=False, debug=False, num_devices=8)
    xT = nc.dram_tensor("xT", [D, L], bf16, kind="ExternalInput")
    wall = nc.dram_tensor("wall", [P, 4 * D], bf16, kind="ExternalInput")
    bqk = nc.dram_tensor("bqk", [P, 2], f32, kind="ExternalInput")
    out = nc.dram_tensor("out", [L, D], bf16, kind="ExternalOutput")

    with tile.TileContext(nc) as tc:
        with (
            tc.tile_pool(name="const", bufs=1) as constp,
            tc.tile_pool(name="stream", bufs=3) as streamp,
            tc.tile_pool(name="expb", bufs=6) as expp,
            tc.tile_pool(name="small", bufs=4) as smallp,
            tc.tile_pool(name="ps_big", bufs=3, space="PSUM") as ps_big,
            tc.tile_pool(name="ps_med", bufs=3, space="PSUM") as ps_med,
            tc.tile_pool(name="ps_av", bufs=2, space="PSUM") as ps_av,
        ):
            # ---------- constants / persistent buffers
            ident = constp.tile([P, P], bf16, tag="ident")
            make_identity(nc, ident[:])
            ones_col = constp.tile([P, 1], bf16, tag="ones")
            nc.vector.memset(ones_col[:], 1.0)
            ones_row_f = constp.tile([1, 64], f32, tag="onesrf")
            nc.vector.memset(ones_row_f[:], 1.0)
            ones_row = constp.tile([1, 64], f32r, tag="onesr")
            nc.vector.tensor_copy(ones_row[:], ones_row_f[:])

            # DMA order feeds quad 0's q-chain first: wq, x0[0:4], wk/wv,
            # x0[4:8], then the rest arrives under compute.
            wq_r = constp.tile([P, D], bf16, tag="wq_r")
            wk_r = constp.tile([P, D], bf16, tag="wk_r")
            wv_r = constp.tile([P, D], bf16, tag="wv_r")
            wo_r = constp.tile([P, D], bf16, tag="wo_r")
            bqk_t = constp.tile([P, 2], f32, tag="bqk")

            xq = [[None] * 8 for _ in range(4)]

            def load_quad_x(quad, kts):
                for kt in kts:
                    t = constp.tile([P, 1024], bf16, tag=f"x{quad}_{kt}")
                    nc.sync.dma_start(
                        t[:], xT[kt * P:(kt + 1) * P, quad * 1024:(quad + 1) * 1024]
                    )
                    xq[quad][kt] = t

            # weights first: the interleaved q/k/v chains touch all three
            # weight tiles at kt=0, so any late weight stalls the in-order
            # PE queue; x tiles then stream at ~PE consumption rate.
            nc.sync.dma_start(wq_r[:], wall[:, 0:D])
            nc.sync.dma_start(wk_r[:], wall[:, D:2 * D])
            nc.sync.dma_start(wv_r[:], wall[:, 2 * D:3 * D])
            load_quad_x(0, range(0, 8))
            nc.sync.dma_start(bqk_t[:], bqk[:])
            load_quad_x(3, range(8))
            nc.sync.dma_start(wo_r[:], wall[:, 3 * D:4 * D])
            load_quad_x(1, range(8))
            load_quad_x(2, range(8))

            qT = constp.tile([P, L], bf16, tag="qT")
            kT = constp.tile([P, L], bf16, tag="kT")
            vTf = constp.tile([P, L], bf16, tag="vTf")
            vn = constp.tile([P, NB * 130], bf16, tag="vn")
            qg = constp.tile([P, 256], bf16, tag="qg")
            for _b in range(NB):
                nc.vector.tensor_copy(vn[:, _b * 130 + 64:_b * 130 + 65], ones_col[:])
                nc.vector.tensor_copy(vn[:, _b * 130 + 129:_b * 130 + 130], ones_col[:])

            loop_ctx = tc.For_i(0, reps, 1) if reps > 1 else contextlib.nullcontext()
            with loop_ctx:
                _body(nc, tc, mybir, Act, f32, f32r, bf16, AluMult, locals())

    nc.compile()
    return nc


def _body(nc, tc, mybir, Act, f32, f32r, bf16, AluMult, env):
    constp = env["constp"]; streamp = env["streamp"]; expp = env["expp"]; smallp = env["smallp"]
    ps_big = env["ps_big"]; ps_med = env["ps_med"]; ps_av = env["ps_av"]
    ident = env["ident"]; ones_row = env["ones_row"]
    wq_r = env["wq_r"]; wk_r = env["wk_r"]; wv_r = env["wv_r"]; wo_r = env["wo_r"]
    bqk_t = env["bqk_t"]
    qT = env["qT"]; kT = env["kT"]; vTf = env["vTf"]; vn = env["vn"]
    qg = env["qg"]
    xq = env["xq"]
    out = env["out"]

    bq = bqk_t[:, 0:1]
    bk = bqk_t[:, 1:2]

    # ---------- phase A: qkv projections + fused v-transpose, per quad
    def do_quad(quad, subs=(0, 1)):
        # q/k/v chains interleaved per contraction step so cold-start PE
        # demand (~640ns/kt) matches x-tile DMA delivery (~790ns/kt).
        for sub in subs:
            n = quad * 2 + sub
            specs = ((wq_r, qT, bq), (wk_r, kT, bk), (wv_r, vTf, None))
            pps = [
                ps_big.tile([P, 512], f32, tag="psbig", name=f"pp{n}_{i}")
                for i in range(len(specs))
            ]
            for kt in range(8):
                for (wt, _, _), pp in zip(specs, pps):
                    nc.tensor.matmul(
                        pp[:], wt[:, kt * P:(kt + 1) * P],
                        xq[quad][kt][:, sub * 512:(sub + 1) * 512],
                        start=kt == 0, stop=kt == 7,
                    )
            sl = slice(n * 512, (n + 1) * 512)
            for i, ((_, dest, bias), pp) in enumerate(zip(specs, pps)):
                if i == 0:
                    nc.scalar.activation(dest[:, sl], pp[:], Act.Identity, bias=bias)
                elif i == 1:
                    nc.vector.tensor_scalar(
                        dest[:, sl], pp[:], bias, None, mybir.AluOpType.add
                    )
                else:
                    nc.vector.tensor_copy(dest[:, sl], pp[:])
            for b in range(8 * quad + 4 * sub, 8 * quad + 4 * sub + 4):
                pst = ps_av.tile([P, P], bf16, tag="psav", name=f"pst{b}")
                nc.tensor.transpose(pst[:], vTf[:, b * P:(b + 1) * P], ident[:])
                base = b * 130
                nc.vector.tensor_copy(vn[:, base:base + 64], pst[:, 0:64])
                nc.vector.tensor_copy(vn[:, base + 65:base + 129], pst[:, 64:128])

    def vslice(blk, h):
        return vn[:, blk * 130 + h * 65: blk * 130 + (h + 1) * 65]

    def normalize_emit(psumT, lo, hi, dests):
        # psumT [65, W+]: rows 0:64 = unnormalized outT, row 64 = l.
        # dests: list of (seg_lo, seg_hi, dest_ap) splitting [lo, hi).
        W = hi - lo
        linv = smallp.tile([1, 512], f32r, tag="linv")
        with nc.allow_low_precision(reason="f32r has near-f32 mantissa here"):
            nc.vector.reciprocal(linv[0:1, 0:W], psumT[64:65, lo:hi])
        psb = ps_med.tile([64, 512], f32, tag="psmed")
        nc.tensor.matmul(psb[0:64, 0:W], ones_row[:], linv[0:1, 0:W],
                         start=True, stop=True)
        bsb = smallp.tile([64, 512], bf16, tag="bsb")
        nc.vector.tensor_copy(bsb[0:64, 0:W], psb[0:64, 0:W])
        for slo, shi, dest in dests:
            nc.vector.tensor_tensor(
                dest, psumT[0:64, slo:shi], bsb[0:64, slo - lo:shi - lo], AluMult
            )

    # ---------- global qtiles (0 and 24): attend to all 32 blocks.
    # Heads interleaved per kblock-pair; prev chunk's deferred out-proj
    # fills PE while the exps cook. Normalized results are written straight
    # into chunk 0's and chunk 6's otr tiles (their col-0 tiles).
    def do_global(prev=None):
        pgs = {}
        for h in (0, 1):
            pgs[h] = ps_av.tile([65, 256], f32, tag="psav", name=f"pg{h}")
        for kb2 in range(NB // 2):
            egh = {}
            for h in (0, 1):
                hs = slice(h * 64, (h + 1) * 64)
                psg = ps_med.tile([P, 512], f32, tag="psmed", name=f"psgg{kb2}_{h}")
                for half in (0, 1):
                    kb = 2 * kb2 + half
                    nc.tensor.matmul(
                        psg[:, half * 256:(half + 1) * 256],
                        kT[hs, kb * P:(kb + 1) * P], qg[hs, :],
                        start=True, stop=True,
                    )
                eg = expp.tile([P, 512], bf16, tag="exp", name=f"egg{kb2}_{h}")
                nc.scalar.activation(eg[:], psg[:], Act.Exp, scale=SCALE)
                egh[h] = eg
            if prev is not None and kb2 < 4:
                outproj_half(prev, (kb2,))
            for h in (0, 1):
                for half in (0, 1):
                    kb = 2 * kb2 + half
                    nc.tensor.matmul(
                        pgs[h][:], vslice(kb, h),
                        egh[h][:, half * 256:(half + 1) * 256],
                        start=kb == 0, stop=kb == NB - 1,
                    )
        for h in (0, 1):
            hsl = slice(h * 64, (h + 1) * 64)
            normalize_emit(pgs[h], 0, 256, [
                (0, 128, otrs[0][0][hsl, 0:128]),
                (128, 256, otrs[6][0][hsl, 0:128]),
            ])

    # ---------- out-projection, emitted deferred (software-pipelined)
    otrs = {}

    def outproj_half(c, ts):
        otr, osb = otrs[c]
        for t in ts:
            for half in (0, 1):
                pso = ps_big.tile([P, 512], f32, tag="psbig", name=f"pso{c}_{t}_{half}")
                nc.tensor.matmul(
                    pso[:], otr[:, t * P:(t + 1) * P],
                    wo_r[:, half * 512:(half + 1) * 512],
                    start=True, stop=True,
                )
                dsl = osb[:, t * 1024 + half * 512: t * 1024 + (half + 1) * 512]
                if half == 0:
                    nc.vector.tensor_copy(dsl, pso[:])
                else:
                    nc.scalar.activation(dsl, pso[:], Act.Identity)
            if c in (0, 6):  # tile-0 lands at the tail: stream per-tile DMAs
                j = 4 * c + t
                # sync HWDGE (not Pool SWDGE) — ~700ns less setup on the
                # drain-critical final transfers
                nc.sync.dma_start(
                    out[j * P:(j + 1) * P, :], osb[:, t * 1024:(t + 1) * 1024]
                )
        if ts[-1] == 3 and c not in (0, 6):
            dram_ap = out[c * 512:(c + 1) * 512, :].rearrange(
                "(j p) d -> p j d", j=4, p=P
            )
            sbuf_ap = osb[:].rearrange("p (j d) -> p j d", j=4)
            nc.gpsimd.dma_start(dram_ap, sbuf_ap)

    # ---------- chunk loop: 8 chunks of 512 q-cols (4 qtiles each).
    # Emission order inside: both heads' scores+exps first (gives Act lead
    # time), prev chunk's out-proj fills PE while exps cook, then AVs, then
    # normalizes — so PE never queues behind a DVE-bound normalize.
    def do_chunk(c, prev=None, prev_ts=((0, 1), (2, 3))):
        otr = smallp.tile([P, 512], bf16, tag="otr", name=f"otr{c}", bufs=5)
        osb = streamp.tile([P, 4096], bf16, tag="osb", name=f"osb{c}", bufs=5)
        otrs[c] = (otr, osb)
        glob_in_chunk = [g for g in GLOB if g // 4 == c]
        lo = 128 if glob_in_chunk else 0
        qts = [4 * c + i for i in range(4) if (4 * c + i) not in GLOB]
        egs = {}
        eds = {}
        for h in (0, 1):
            hs = slice(h * 64, (h + 1) * 64)
            for g in GLOB:
                psg = ps_med.tile([P, 512], f32, tag="psmed", name=f"psg{c}_{h}_{g}")
                nc.tensor.matmul(
                    psg[:], kT[hs, g * P:(g + 1) * P],
                    qT[hs, c * 512:(c + 1) * 512],
                    start=True, stop=True,
                )
                eg = expp.tile([P, 512], bf16, tag="exp", name=f"eg{c}_{h}_{g}")
                nc.scalar.activation(eg[:], psg[:], Act.Exp, scale=SCALE)
                egs[(h, g)] = eg
            psd = ps_med.tile([P, 512], f32, tag="psmed", name=f"psd{c}_{h}")
            for idx, j in enumerate(qts):
                nc.tensor.matmul(
                    psd[:, idx * P:(idx + 1) * P],
                    kT[hs, j * P:(j + 1) * P], qT[hs, j * P:(j + 1) * P],
                    start=True, stop=True, skip_group_check=True,
                )
            wd = len(qts) * P
            ed = expp.tile([P, 512], bf16, tag="exp", name=f"ed{c}_{h}")
            nc.scalar.activation(ed[:, 0:wd], psd[:, 0:wd], Act.Exp, scale=SCALE)
            eds[h] = ed
        pcs = {}
        for h in (0, 1):
            pc = ps_av.tile([65, 512], f32, tag="psav", name=f"pc{c}_{h}")
            nc.tensor.matmul(pc[:, lo:512], vslice(GLOB[0], h),
                             egs[(h, GLOB[0])][:, lo:512], start=True, stop=False)
            nc.tensor.matmul(pc[:, lo:512], vslice(GLOB[1], h),
                             egs[(h, GLOB[1])][:, lo:512], start=False, stop=True)
            for idx, j in enumerate(qts):
                off = (j - 4 * c) * P
                nc.tensor.matmul(pc[:, off:off + P], vslice(j, h),
                                 eds[h][:, idx * P:(idx + 1) * P],
                                 start=False, stop=True,
                                 skip_group_check=True)  # sub-region accumulate
            pcs[h] = pc
            if prev is not None:
                outproj_half(prev, prev_ts[h])
        for h in (0, 1):
            normalize_emit(
                pcs[h], lo, 512, [(lo, 512, otr[h * 64:(h + 1) * 64, lo:512])]
            )

    # Interleave: quads 0,3 first (global kblocks 0 & 24 live there), then
    # alternate chunk work (Act-heavy) with remaining quads (PE-heavy).
    # Emission: half-quads (PE-heavy, Act-light) sprinkled between chunks
    # (Act-heavy) across the whole timeline. Chunks 0/6 run early — their
    # non-global attention only needs quads 0 and 3; their col-0 out-proj
    # tile is gated on do_global and lands in the tail.
    do_quad(0, subs=(0,))
    do_quad(3, subs=(0,))
    nc.vector.tensor_copy(qg[:, 0:128], qT[:, 0:128])
    nc.vector.tensor_copy(qg[:, 128:256], qT[:, GLOB[1] * P:(GLOB[1] + 1) * P])
    do_quad(0, subs=(1,))
    do_chunk(0)
    do_quad(3, subs=(1,))
    do_chunk(1, prev=0, prev_ts=((1, 2), (3,)))
    do_chunk(7, prev=1)
    do_chunk(6, prev=7)
    do_quad(1, subs=(0,))
    do_chunk(2, prev=6, prev_ts=((1, 2), (3,)))
    do_quad(1, subs=(1,))
    do_chunk(3, prev=2)
    do_quad(2, subs=(0,))
    do_chunk(4, prev=3)
    do_quad(2, subs=(1,))
    do_chunk(5, prev=4)
    do_global(prev=5)
    outproj_half(0, (0,))
    outproj_half(6, (0,))


def _get_nc(reps=1):
    key = ("nc", reps)
    if key not in _CACHE:
        _CACHE[key] = _build_nc(reps)
    return _CACHE[key]


def _prep_inputs(x, w_qkv, b_qkv):
    import ml_dtypes

    bf16 = ml_dtypes.bfloat16
    x2 = np.asarray(x, dtype=np.float32).reshape(L, D)
    xT = np.ascontiguousarray(x2.T).astype(bf16)
    w_qkv = np.asarray(w_qkv, dtype=np.float32)
    b_qkv = np.asarray(b_qkv, dtype=np.float32)

    def tile_w(w_slice):
        wt = w_slice.T
        return np.ascontiguousarray(
            wt.reshape(8, P, P).transpose(1, 0, 2).reshape(P, D)
        )

    maps = []
    for c in range(8):
        a = 2 * c * HD
        b = a + 2 * HD
        maps.append({
            "xT": xT,
            "_wq": tile_w(w_qkv[a:b, :]),
            "_wk": tile_w(w_qkv[D + a:D + b, :]),
            "_wv": tile_w(w_qkv[2 * D + a:2 * D + b, :]),
            "bqk": np.ascontiguousarray(
                np.stack([b_qkv[a:b], b_qkv[D + a:D + b]], axis=1)
            ).astype(np.float32),
        })
    return maps


def _finish_maps(maps, w_out):
    import ml_dtypes

    bf16 = ml_dtypes.bfloat16
    w_out = np.asarray(w_out, dtype=np.float32)
    for c in range(8):
        a = 2 * c * HD
        b = a + 2 * HD
        wo = np.ascontiguousarray(w_out[:, a:b].T)
        wall = np.concatenate(
            [maps[c].pop("_wq"), maps[c].pop("_wk"), maps[c].pop("_wv"), wo],
            axis=1,
        ).astype(bf16)
        maps[c]["wall"] = np.ascontiguousarray(wall)
    return maps


def kernel(x, w_qkv, b_qkv, w_out, b_out):
    from concourse.bass_utils import run_bass_kernel_spmd

    x = np.asarray(x, dtype=np.float32)
    w_qkv = np.asarray(w_qkv, dtype=np.float32)
    b_qkv = np.asarray(b_qkv, dtype=np.float32)
    w_out = np.asarray(w_out, dtype=np.float32)
    b_out = np.asarray(b_out, dtype=np.float32)

    nc = _get_nc()
    maps = _finish_maps(_prep_inputs(x, w_qkv, b_qkv), w_out)

    res = run_bass_kernel_spmd(nc, maps, core_ids=list(range(8)))

    total = res.results[0]["out"].astype(np.float32)
    for c in range(1, 8):
        total += res.results[c]["out"].astype(np.float32)
    const_row = b_qkv[2 * D:3 * D] @ w_out.T + b_out
    total += const_row[None, :]
    return total.reshape(x.shape).astype(np.float32)
